# revision 24
# baseline (speedup 1.0000x reference)
"""Trainium2 Bass kernel for the FastNeuron scatter-memory module.

Strategy: pure data-parallel over batch B=8 -> one batch element per
NeuronCore, identical SPMD program on 8 cores.

Per-core program (batch element b):
  pass 1: stream h (4096x2560) once, reduce over S via PE ones-matmul
          -> h_avg.
  middle: the whole surprise/write/read/projection chain as small
          matvecs on PE (column-form: K on partitions, M in chunks of
          128 columns), activations on ACT, elementwise on DVE.
          Weights stream from HBM in chunk-major layout.
  pass 2: stream h again, fused  h_new = LN(h + g*output)  with
          DVE add + bn_stats, ACT normalize, DMA out.

All I/O is f32.  Everything is DMA-bound by design (~206 MB/core).
"""

import contextlib
import math

import numpy as np

import concourse.bacc as bacc
import concourse.bass as bass
import concourse.tile as tile
from concourse import mybir

F32 = mybir.dt.float32
AF = mybir.ActivationFunctionType
OP = mybir.AluOpType

B, S_FULL, D = 8, 4096, 2560
R, DQ, DV, DP, DC, DH = 64, 128, 512, 128, 128, 384
MAX_NORM = 10.0
DCOL = D // 128  # 20 columns in col128 layout
NCH = D // 512   # 5 chunks of 512

# (name, K, M) for every linear layer's weight
_WSPECS = {
    'sp1': (D + DC, DH), 'sp2': (DH, D),
    'su1': (D + DC, DH), 'su2': (DH, 1),
    'wk1': (D + 1 + DC, DH), 'wk2': (DH, R),
    'wv1': (D + 1 + DC, DH), 'wv2': (DH, D),
    'lr1': (1 + DC, DH // 2), 'lr2': (DH // 2, 1),
    'rq': (D + DC, DQ),
    'vup': (DV, D),
    'pd1': (D + 1 + D + DC, DH), 'pd2': (DH, D + DP),
    'pu1': (D + 1 + D + DC, DH), 'pu2': (DH, DP + D),
    'pl1': (1 + DC, 64), 'pl2': (64, 1),
    'g1': (D + D + DC, DH), 'g2': (DH, 1),
}

_WMAXFREE = 4032  # elements per partition per weight-stage tile


def _cdiv(a, b):
    return (a + b - 1) // b


def build(S=S_FULL, ln_affine=False, sln_affine=False):
    """Build the per-core Bass program."""
    assert S % 128 == 0
    ST = S // 128

    nc = bacc.Bacc("TRN2", target_bir_lowering=False, debug=False)

    h_in = nc.dram_tensor("h", [S, D], F32, kind="ExternalInput").ap()
    memA_in = nc.dram_tensor("mem_A", [D, R], F32, kind="ExternalInput").ap()
    wdm_in = nc.dram_tensor("W_down_mod", [D, DP], F32, kind="ExternalInput").ap()
    wum_in = nc.dram_tensor("W_up_mod", [DP, D], F32, kind="ExternalInput").ap()
    pha_in = nc.dram_tensor("pha_col", [128, DCOL], F32, kind="ExternalInput").ap()
    ctx_in = nc.dram_tensor("ctx_col", [128, 1], F32, kind="ExternalInput").ap()
    ident_in = nc.dram_tensor("ident", [128, 128], F32, kind="ExternalInput").ap()

    wdram = {}
    for name, (K, M) in _WSPECS.items():
        wdram[name + '_w'] = nc.dram_tensor(name + '_w', [K, M], F32,
                                            kind="ExternalInput").ap()
        wdram[name + '_b'] = nc.dram_tensor(name + '_b', [1, M], F32,
                                            kind="ExternalInput").ap()
    for name, shape in [('W_K', [D, DQ]), ('W_V', [D, DV]),
                        ('W_down_base', [D, DP]), ('W_up_base', [DP, D])]:
        wdram[name] = nc.dram_tensor(name, shape, F32, kind="ExternalInput").ap()
    if sln_affine:
        wdram['sln_w'] = nc.dram_tensor('sln_w', [1, D], F32, kind="ExternalInput").ap()
        wdram['sln_b'] = nc.dram_tensor('sln_b', [1, D], F32, kind="ExternalInput").ap()
    if ln_affine:
        wdram['ln_w'] = nc.dram_tensor('ln_w', [1, D], F32, kind="ExternalInput").ap()
        wdram['ln_b'] = nc.dram_tensor('ln_b', [1, D], F32, kind="ExternalInput").ap()

    h_out = nc.dram_tensor("h_new", [S, D], F32, kind="ExternalOutput").ap()
    mem_out = nc.dram_tensor("mem_out", [D, R], F32, kind="ExternalOutput").ap()
    wdn_out = nc.dram_tensor("W_down_new", [D, DP], F32, kind="ExternalOutput").ap()
    wun_out = nc.dram_tensor("W_up_new", [DP, D], F32, kind="ExternalOutput").ap()

    with tile.TileContext(nc) as tc:
        _body(nc, tc, S, ST, h_in, memA_in, wdm_in, wum_in, pha_in, ctx_in,
              ident_in, wdram, h_out, mem_out, wdn_out, wun_out,
              ln_affine, sln_affine)

    nc.compile()
    return nc


def _body(nc, tc, S, ST, h_in, memA_in, wdm_in, wum_in, pha_in, ctx_in,
          ident_in, wdram, h_out, mem_out, wdn_out, wun_out,
          ln_affine, sln_affine):
    stack = contextlib.ExitStack()
    persist = stack.enter_context(tc.tile_pool(name="persist", bufs=1))
    hpool = stack.enter_context(tc.tile_pool(name="hpool", bufs=3))
    wstream = stack.enter_context(tc.tile_pool(name="wstream", bufs=3))
    wsmall = stack.enter_context(tc.tile_pool(name="wsmall", bufs=3))
    brow = stack.enter_context(tc.tile_pool(name="brow", bufs=2))
    rows = stack.enter_context(tc.tile_pool(name="rows", bufs=2))
    vecs = stack.enter_context(tc.tile_pool(name="vecs", bufs=2))
    bigscr = stack.enter_context(tc.tile_pool(name="bigscr", bufs=3))
    pscol = stack.enter_context(tc.tile_pool(name="pscol", bufs=3, space="PSUM"))
    psrow = stack.enter_context(tc.tile_pool(name="psrow", bufs=1, space="PSUM"))

    # ---------------- persistent small tiles ----------------
    ident = persist.tile([128, 128], F32)
    nc.sync.dma_start(out=ident, in_=ident_in)
    one = ident[0:1, 0:1]  # scalar 1.0

    ones_col = persist.tile([128, 1], F32)
    nc.vector.memset(ones_col, 1.0)
    ones_row = persist.tile([1, 128], F32)
    nc.vector.memset(ones_row, 1.0)

    ctx_col = persist.tile([128, 1], F32)
    nc.sync.dma_start(out=ctx_col, in_=ctx_in)
    pha_col = persist.tile([128, DCOL], F32)
    nc.sync.dma_start(out=pha_col, in_=pha_in)

    eps5 = persist.tile([128, 1], F32)
    nc.vector.memset(eps5, 1e-5)
    eps8 = persist.tile([1, 1], F32)
    nc.vector.memset(eps8, 1e-8)
    neg5 = persist.tile([1, 1], F32)
    nc.vector.memset(neg5, -5.0)

    # ---------------- helpers ----------------
    def load_chunks(w_ap, r0, nk, M, krem=0, pool=None, tag="wstream"):
        """Load rows [r0, r0+nk*128+krem) of w chunk-major into one or more
        sbuf tiles.  Returns list of (tile, nchunks, kremainder)."""
        pool = pool or wstream
        G = max(1, _WMAXFREE // M)
        segs = []
        c0 = 0
        while c0 < nk:
            g = min(G, nk - c0)
            t = pool.tile([128, g, M], F32, tag=tag)
            src = w_ap[r0 + c0 * 128: r0 + (c0 + g) * 128, :]
            nc.sync.dma_start(out=t, in_=src.rearrange("(c p) m -> p c m", p=128))
            segs.append((t, g, 0))
            c0 += g
        if krem:
            t = pool.tile([128, 1, M], F32, tag=tag)
            src = w_ap[r0 + nk * 128: r0 + nk * 128 + krem, :]
            nc.sync.dma_start(out=t[0:krem, 0, :], in_=src)
            segs.append((t, 1, krem))
        return segs

    def load_small(w_ap, r0, nk, M, krem=0):
        return load_chunks(w_ap, r0, nk, M, krem=krem, pool=wsmall, tag="wsm")

    def load_row(w_ap, r0, M, pool, tag):
        t = pool.tile([1, M], F32, tag=tag)
        nc.sync.dma_start(out=t, in_=w_ap[r0:r0 + 1, :])
        return t

    def load_bias(name):
        return load_row(wdram[name + '_b'], 0, _WSPECS[name][1], brow, "bias")

    def _chunk_ops(parts):
        """Flatten parts into per-K-chunk (wt_slice_fn, x_slice) pairs,
        ordered so weight segments are consumed strictly sequentially."""
        ops = []
        for (x, segs, kind) in parts:
            if kind == 's':
                def fn(lo, w, wt=segs):
                    return wt[0:1, lo:lo + w]
                ops.append((fn, x[0:1, 0:1]))
                continue
            xc = 0
            for (t, g, kr) in segs:
                if kr:
                    def fn(lo, w, t=t, kr=kr):
                        return t[0:kr, 0, lo:lo + w]
                    ops.append((fn, x[0:kr, xc:xc + 1]))
                    xc += 1
                else:
                    for c in range(g):
                        def fn(lo, w, t=t, c=c):
                            return t[:, c, lo:lo + w]
                        ops.append((fn, x[:, xc:xc + 1]))
                        xc += 1
        return ops

    def mv_col(parts, M, bias_row, act, out_pool, tag, m_off=0, act_scale=1.0):
        """Column-form matvec -> sbuf col tile [128, ceil(M/128)].
        K-chunk-major loop so weight segments stream through few slots."""
        n_mc = _cdiv(M, 128)
        widths = [min(128, M - mc * 128) for mc in range(n_mc)]
        ps = pscol.tile([128, n_mc], F32, tag="colps")
        ops = _chunk_ops(parts)
        total = (len(ops) + (1 if bias_row is not None else 0)) * n_mc
        # start=True zeroes the WHOLE 2KB psum bank, so only the very first
        # matmul of the matvec starts; later columns accumulate onto zeros.
        idx = 0
        for (fn, xs) in ops:
            for mc in range(n_mc):
                lo, w = m_off + mc * 128, widths[mc]
                nc.tensor.matmul(ps[0:w, mc:mc + 1], fn(lo, w), xs,
                                 start=(idx == 0), stop=(idx == total - 1),
                                 skip_group_check=True)
                idx += 1
        if bias_row is not None:
            for mc in range(n_mc):
                lo, w = m_off + mc * 128, widths[mc]
                nc.tensor.matmul(ps[0:w, mc:mc + 1],
                                 bias_row[0:1, lo:lo + w], one,
                                 start=(idx == 0), stop=(idx == total - 1),
                                 skip_group_check=True)
                idx += 1
        out = out_pool.tile([128, n_mc], F32, tag=tag)
        if all(w == 128 for w in widths):
            nc.scalar.activation(out, ps, act, scale=act_scale)
        else:
            for mc in range(n_mc):
                w = widths[mc]
                nc.scalar.activation(out[0:w, mc:mc + 1], ps[0:w, mc:mc + 1],
                                     act, scale=act_scale)
        return out

    def mv_row(parts, M, bias_row, act, tag, m_off=0, act_scale=1.0):
        """Row-form matvec -> sbuf row tile [1, M]."""
        n_nc = _cdiv(M, 512)
        widths = [min(512, M - i * 512) for i in range(n_nc)]
        ps = psrow.tile([1, M], F32, tag="rowps")
        ops = _chunk_ops(parts)
        total = len(ops) + (1 if bias_row is not None else 0)
        for i, (fn, xs) in enumerate(ops):
            for ncI in range(n_nc):
                lo, w = m_off + ncI * 512, widths[ncI]
                nc.tensor.matmul(ps[0:1, ncI * 512: ncI * 512 + w],
                                 xs, fn(lo, w),
                                 start=(i == 0), stop=(i == total - 1),
                                 skip_group_check=True)
        if bias_row is not None:
            for ncI in range(n_nc):
                lo, w = m_off + ncI * 512, widths[ncI]
                nc.tensor.matmul(ps[0:1, ncI * 512: ncI * 512 + w],
                                 one, bias_row[0:1, lo:lo + w],
                                 start=(total == 1), stop=True,
                                 skip_group_check=True)
        out = rows.tile([1, M], F32, tag=tag)
        nc.scalar.activation(out, ps, act, scale=act_scale)
        return out

    def col_to_row(col, ncols, tag, scale=1.0):
        ps = psrow.tile([1, ncols * 128], F32, tag="rowps")
        for c in range(ncols):
            # 4 chunks of 512B per 2KB bank: start only on each bank's first
            nc.tensor.matmul(ps[0:1, c * 128:(c + 1) * 128], col[:, c:c + 1],
                             ident, start=(c % 4 == 0), stop=(c == ncols - 1),
                             skip_group_check=True)
        out = rows.tile([1, ncols * 128], F32, tag=tag)
        nc.scalar.activation(out, ps, AF.Copy, scale=scale)
        return out

    def row_to_col(row, ncols, out_pool, tag, scale=1.0):
        ps = pscol.tile([128, ncols], F32, tag="colps")
        for c in range(ncols):
            nc.tensor.matmul(ps[:, c:c + 1], row[0:1, c * 128:(c + 1) * 128],
                             one, start=(c == 0), stop=(c == ncols - 1),
                             skip_group_check=True)
        out = out_pool.tile([128, ncols], F32, tag=tag)
        nc.scalar.activation(out, ps, AF.Copy, scale=scale)
        return out

    def bcast(src, n, out_pool, tag):
        """[1, n] -> [128, n] via K=1 PE outer product with a ones column."""
        out = out_pool.tile([128, n], F32, tag=tag)
        for j in range(0, n, 512):
            w = min(512, n - j)
            ps = pscol.tile([128, 512], F32, tag="colps")
            nc.tensor.matmul(ps[:, 0:w], ones_row, src[0:1, j:j + w],
                             start=True, stop=True)
            nc.scalar.activation(out[:, j:j + w], ps[:, 0:w], AF.Copy)
        return out

    # =========================================================
    # PASS 1: sum of h over S
    # =========================================================
    ps_hsum = psrow.tile([1, D], F32, tag="rowps")
    for t in range(ST):
        h_t = hpool.tile([128, D], F32, tag="htile")
        nc.sync.dma_start(out=h_t, in_=h_in[t * 128:(t + 1) * 128, :])
        for c in range(NCH):
            nc.tensor.matmul(ps_hsum[0:1, c * 512:(c + 1) * 512], ones_col,
                             h_t[:, c * 512:(c + 1) * 512],
                             start=(t == 0), stop=(t == ST - 1),
                             skip_group_check=True)

    h_avg_row = rows.tile([1, D], F32, tag="rowbuf")
    nc.scalar.activation(h_avg_row, ps_hsum, AF.Copy, scale=1.0 / S)
    h_avg = row_to_col(h_avg_row, DCOL, persist, "havg")

    # =========================================================
    # MIDDLE
    # =========================================================
    # ---- surprise prediction ----
    sp1_w = load_chunks(wdram['sp1_w'], 0, DCOL, DH)
    sp1_wc = load_small(wdram['sp1_w'], D, 1, DH)
    hid = mv_col([(pha_col, sp1_w, 'c'), (ctx_col, sp1_wc, 'c')],
                 DH, load_bias('sp1'), AF.Gelu, vecs, "hid")
    sp2_w = load_chunks(wdram['sp2_w'], 0, 3, D)
    pred = mv_col([(hid, sp2_w, 'c')], D, load_bias('sp2'), AF.Copy,
                  vecs, "pred")

    err = vecs.tile([128, DCOL], F32, tag="err")
    nc.vector.tensor_sub(err, h_avg, pred)

    su1_w = load_chunks(wdram['su1_w'], 0, DCOL, DH)
    su1_wc = load_small(wdram['su1_w'], D, 1, DH)
    hid2 = mv_col([(err, su1_w, 'c'), (ctx_col, su1_wc, 'c')],
                  DH, load_bias('su1'), AF.Gelu, vecs, "hid")
    su2_w = load_small(wdram['su2_w'], 0, 3, 1)
    surprise = mv_col([(hid2, su2_w, 'c')], 1, load_bias('su2'),
                      AF.Sigmoid, persist, "surprise")
    surp = surprise[0:1, 0:1]

    # ---- write key / value ----
    wk1_w = load_chunks(wdram['wk1_w'], 0, DCOL, DH)
    wk1_ws = load_row(wdram['wk1_w'], D, DH, wsmall, "wrow")
    wk1_wc = load_small(wdram['wk1_w'], D + 1, 1, DH)
    wi = [(h_avg, wk1_w, 'c'), (surp, wk1_ws, 's'), (ctx_col, wk1_wc, 'c')]
    hidk = mv_col(wi, DH, load_bias('wk1'), AF.Gelu, vecs, "hidk")
    wk2_w = load_small(wdram['wk2_w'], 0, 3, R)
    wkey_row = mv_row([(hidk, wk2_w, 'c')], R, load_bias('wk2'),
                      AF.Copy, "rowbuf")

    wv1_w = load_chunks(wdram['wv1_w'], 0, DCOL, DH)
    wv1_ws = load_row(wdram['wv1_w'], D, DH, wsmall, "wrow")
    wv1_wc = load_small(wdram['wv1_w'], D + 1, 1, DH)
    wiv = [(h_avg, wv1_w, 'c'), (surp, wv1_ws, 's'), (ctx_col, wv1_wc, 'c')]
    hidv = mv_col(wiv, DH, load_bias('wv1'), AF.Gelu, vecs, "hidv")
    wv2_w = load_chunks(wdram['wv2_w'], 0, 3, D)
    wval = mv_col([(hidv, wv2_w, 'c')], D, load_bias('wv2'), AF.Copy,
                  vecs, "wval")

    # ---- lr ----
    lr1_ws = load_row(wdram['lr1_w'], 0, DH // 2, wsmall, "wrow")
    lr1_wc = load_small(wdram['lr1_w'], 1, 1, DH // 2)
    hidl = mv_col([(surp, lr1_ws, 's'), (ctx_col, lr1_wc, 'c')],
                  DH // 2, load_bias('lr1'), AF.Gelu, vecs, "hidl")
    lr2_w = load_small(wdram['lr2_w'], 0, 1, 1, krem=64)
    # softplus(x) = ln(1 + exp(x)) -- no native Softplus LUT set in walrus
    lr_e = mv_col([(hidl, lr2_w, 'c')], 1, load_bias('lr2'),
                  AF.Exp, vecs, "lrsp")
    lr = vecs.tile([1, 1], F32, tag="lr")
    nc.vector.tensor_scalar_add(lr, lr_e[0:1, 0:1], 1.0)
    nc.scalar.activation(lr, lr, AF.Ln)
    nc.vector.tensor_single_scalar(lr, lr, 0.1, op=OP.min)

    # ---- mem = mem_A + lr * wval wkey^T ; clip by global norm ----
    lr_b = bcast(lr, 1, vecs, "lrb")
    lrwval = vecs.tile([128, DCOL], F32, tag="lrwval")
    nc.vector.tensor_scalar_mul(lrwval, wval, lr_b)
    wkey_b = bcast(wkey_row, R, vecs, "wkeyb")

    memA = bigscr.tile([128, DCOL, R], F32, tag="bigscr")
    nc.sync.dma_start(out=memA, in_=memA_in.rearrange("(c p) r -> p c r", p=128))
    mem_pre = persist.tile([128, DCOL, R], F32, tag="mempre")
    for c in range(DCOL):
        nc.vector.scalar_tensor_tensor(mem_pre[:, c, :], wkey_b,
                                       lrwval[:, c:c + 1], memA[:, c, :],
                                       op0=OP.mult, op1=OP.add)
    sq_acc = vecs.tile([128, 1], F32, tag="sqacc")
    sq_scr = bigscr.tile([128, DCOL * R], F32, tag="bigscr")
    nc.scalar.activation(sq_scr, mem_pre.rearrange("p c r -> p (c r)"),
                         AF.Square, accum_out=sq_acc)
    ps_n = pscol.tile([128, 1], F32, tag="colps")
    nc.tensor.matmul(ps_n[0:1, 0:1], sq_acc, ones_col, start=True, stop=True)
    normv = vecs.tile([1, 1], F32, tag="normv")
    nc.scalar.activation(normv, ps_n[0:1, 0:1], AF.Sqrt, bias=eps8)
    rec = vecs.tile([1, 1], F32, tag="recn")
    nc.vector.reciprocal(rec, normv)
    mscale = vecs.tile([1, 1], F32, tag="mscale")
    nc.vector.tensor_scalar(mscale, rec, MAX_NORM, 1.0, op0=OP.mult, op1=OP.min)
    ms_b = bcast(mscale, 1, vecs, "msb")
    nc.vector.tensor_scalar_mul(mem_pre.rearrange("p c r -> p (c r)"),
                                mem_pre.rearrange("p c r -> p (c r)"), ms_b)
    nc.sync.dma_start(out=mem_out.rearrange("(c p) r -> p c r", p=128),
                      in_=mem_pre)

    # ---- slots = LN(mem^T) ----
    memT = bigscr.tile([R, D], F32, tag="bigscr")
    for c in range(DCOL):
        psT = pscol.tile([128, 128], F32, tag="colps")
        nc.tensor.matmul(psT[0:R, :], mem_pre[:, c, :], ident,
                         start=True, stop=True)
        nc.scalar.activation(memT[:, c * 128:(c + 1) * 128], psT[0:R, :], AF.Copy)
    stats = vecs.tile([R, NCH, 6], F32, tag="slnstats")
    for c in range(NCH):
        nc.vector.bn_stats(stats[:, c, :], memT[:, c * 512:(c + 1) * 512])
    mv_ = vecs.tile([R, 2], F32, tag="slnmv")
    nc.vector.bn_aggr(mv_, stats)
    s_rstd = vecs.tile([R, 1], F32, tag="srstd")
    nc.scalar.activation(s_rstd, mv_[:, 1:2], AF.Sqrt, bias=eps5[0:R, 0:1])
    nc.vector.reciprocal(s_rstd, s_rstd)
    s_nbias = vecs.tile([R, 1], F32, tag="snbias")
    nc.vector.scalar_tensor_tensor(s_nbias, mv_[:, 0:1], -1.0, s_rstd,
                                   op0=OP.mult, op1=OP.mult)
    slots = bigscr.tile([R, D], F32, tag="bigscr")
    nc.scalar.activation(slots, memT, AF.Identity, bias=s_nbias, scale=s_rstd)
    if sln_affine:
        slw = bcast(load_row(wdram['sln_w'], 0, D, brow, "bias"), D,
                    bigscr, "bigscr")
        slb = bcast(load_row(wdram['sln_b'], 0, D, brow, "bias"), D,
                    bigscr, "bigscr")
        nc.vector.tensor_mul(slots, slots, slw[0:R, :])
        nc.vector.tensor_add(slots, slots, slb[0:R, :])

    slotsT = persist.tile([128, DCOL, R], F32, tag="slotsT")
    for c in range(DCOL):
        psT = pscol.tile([128, R], F32, tag="colps")
        nc.tensor.matmul(psT, slots[:, c * 128:(c + 1) * 128], ident[0:R, 0:R],
                         start=True, stop=True)
        nc.scalar.activation(slotsT[:, c, :], psT, AF.Copy)

    # ---- attention read ----
    wk_w = load_chunks(wdram['W_K'], 0, DCOL, DQ)
    ps_k = pscol.tile([128, R], F32, tag="colps")
    ci = 0
    for (t_, g, kr) in wk_w:
        for c in range(g):
            nc.tensor.matmul(ps_k, t_[:, c, :], slotsT[:, ci, :],
                             start=(ci == 0), stop=(ci == DCOL - 1))
            ci += 1
    keysT = vecs.tile([128, R], F32, tag="keysT")
    nc.scalar.activation(keysT, ps_k, AF.Copy, scale=1.0 / math.sqrt(DQ))

    rq_w = load_chunks(wdram['rq_w'], 0, DCOL, DQ)
    rq_wc = load_small(wdram['rq_w'], D, 1, DQ)
    query = mv_col([(h_avg, rq_w, 'c'), (ctx_col, rq_wc, 'c')],
                   DQ, load_bias('rq'), AF.Copy, vecs, "query")

    ps_s = psrow.tile([1, R], F32, tag="rowps")
    nc.tensor.matmul(ps_s, query, keysT, start=True, stop=True)
    smax = vecs.tile([1, 1], F32, tag="smax")
    nc.vector.reduce_max(smax, ps_s, axis=mybir.AxisListType.X)
    negmax = vecs.tile([1, 1], F32, tag="negmax")
    nc.vector.tensor_scalar_mul(negmax, smax, -1.0)
    esum = vecs.tile([1, 1], F32, tag="esum")
    attn_row = rows.tile([1, R], F32, tag="rowbuf")
    nc.scalar.activation(attn_row, ps_s, AF.Exp, bias=negmax, accum_out=esum)
    einv = vecs.tile([1, 1], F32, tag="einv")
    nc.vector.reciprocal(einv, esum)
    nc.vector.tensor_scalar_mul(attn_row, attn_row, einv)
    ps_a = pscol.tile([128, 1], F32, tag="colps")
    nc.tensor.matmul(ps_a[0:R, 0:1], attn_row, one, start=True, stop=True)
    attn_col = vecs.tile([R, 1], F32, tag="attncol")
    nc.scalar.activation(attn_col, ps_a[0:R, 0:1], AF.Copy)

    wv_w = load_chunks(wdram['W_V'], 0, DCOL, DV)
    ps_v = pscol.tile([R, DV], F32, tag="colps")
    ci = 0
    for (t_, g, kr) in wv_w:
        for c in range(g):
            nc.tensor.matmul(ps_v, slotsT[:, ci, :], t_[:, c, :],
                             start=(ci == 0), stop=(ci == DCOL - 1))
            ci += 1
    vals = bigscr.tile([R, DV], F32, tag="bigscr")
    nc.scalar.activation(vals, ps_v, AF.Copy)

    ps_mr = pscol.tile([128, DV // 128], F32, tag="colps")
    for dc in range(DV // 128):
        nc.tensor.matmul(ps_mr[:, dc:dc + 1], vals[:, dc * 128:(dc + 1) * 128],
                         attn_col, start=(dc == 0), stop=(dc == DV // 128 - 1),
                         skip_group_check=True)
    mr_dv = vecs.tile([128, DV // 128], F32, tag="mrdv")
    nc.scalar.activation(mr_dv, ps_mr, AF.Copy)

    vup_w = load_chunks(wdram['vup_w'], 0, 4, D)
    mem_read = mv_col([(mr_dv, vup_w, 'c')], D, load_bias('vup'),
                      AF.Copy, persist, "memread")

    # ---- modulated projection ----
    wdb = load_chunks(wdram['W_down_base'], 0, DCOL, DP)
    wdm = bigscr.tile([128, DCOL, DP], F32, tag="bigscr")
    nc.sync.dma_start(out=wdm, in_=wdm_in.rearrange("(c p) q -> p c q", p=128))
    assert len(wdb) == 1
    wdeff = wdb[0][0]  # W_down_eff computed in place in the stream tile
    nc.vector.tensor_add(wdeff.rearrange("p c q -> p (c q)"),
                         wdeff.rearrange("p c q -> p (c q)"),
                         wdm.rearrange("p c q -> p (c q)"))
    ps_dn = pscol.tile([128, 1], F32, tag="colps")
    for c in range(DCOL):
        nc.tensor.matmul(ps_dn, wdeff[:, c, :], mem_read[:, c:c + 1],
                         start=(c == 0), stop=(c == DCOL - 1))
    down = vecs.tile([128, 1], F32, tag="down")
    nc.scalar.activation(down, ps_dn, AF.Gelu)

    wub = bigscr.tile([128, D], F32, tag="bigscr")
    nc.sync.dma_start(out=wub, in_=wdram['W_up_base'])
    wum = bigscr.tile([128, D], F32, tag="bigscr")
    nc.sync.dma_start(out=wum, in_=wum_in)
    nc.vector.tensor_add(wub, wub, wum)  # wub := W_up_eff
    ps_pr = pscol.tile([128, DCOL], F32, tag="colps")
    for c in range(DCOL):
        nc.tensor.matmul(ps_pr[:, c:c + 1], wub[:, c * 128:(c + 1) * 128],
                         down, start=(c == 0), stop=(c == DCOL - 1),
                         skip_group_check=True)
    proj = vecs.tile([128, DCOL], F32, tag="proj")
    nc.scalar.activation(proj, ps_pr, AF.Copy)

    output = persist.tile([128, DCOL], F32, tag="output")
    nc.vector.scalar_tensor_tensor(output, mem_read, 0.5, proj,
                                   op0=OP.mult, op1=OP.add)

    # ---- projection weight write ----
    ws_ = vecs.tile([1, 1], F32, tag="ws")
    nc.scalar.activation(ws_, surp, AF.Sigmoid, bias=neg5, scale=10.0)

    pd1_w = load_chunks(wdram['pd1_w'], 0, DCOL, DH)
    pd1_ws = load_row(wdram['pd1_w'], D, DH, wsmall, "wrow")
    pd1_wm = load_chunks(wdram['pd1_w'], D + 1, DCOL, DH)
    pd1_wc = load_small(wdram['pd1_w'], 2 * D + 1, 1, DH)
    pwi_d = [(h_avg, pd1_w, 'c'), (surp, pd1_ws, 's'),
             (mem_read, pd1_wm, 'c'), (ctx_col, pd1_wc, 'c')]
    hidp = mv_col(pwi_d, DH, load_bias('pd1'), AF.Gelu, vecs, "hidp")
    pd2_w = load_chunks(wdram['pd2_w'], 0, 3, D + DP)
    pd2_b = load_bias('pd2')
    d_pat = mv_col([(hidp, pd2_w, 'c')], D, pd2_b, AF.Copy, vecs, "dpat")
    d_addr_row = mv_row([(hidp, pd2_w, 'c')], DP, pd2_b, AF.Copy,
                        "rowbuf", m_off=D)

    pu1_w = load_chunks(wdram['pu1_w'], 0, DCOL, DH)
    pu1_ws = load_row(wdram['pu1_w'], D, DH, wsmall, "wrow")
    pu1_wm = load_chunks(wdram['pu1_w'], D + 1, DCOL, DH)
    pu1_wc = load_small(wdram['pu1_w'], 2 * D + 1, 1, DH)
    pwi_u = [(h_avg, pu1_w, 'c'), (surp, pu1_ws, 's'),
             (mem_read, pu1_wm, 'c'), (ctx_col, pu1_wc, 'c')]
    hidu = mv_col(pwi_u, DH, load_bias('pu1'), AF.Gelu, vecs, "hidu")
    pu2_w = load_chunks(wdram['pu2_w'], 0, 3, DP + D)
    pu2_b = load_bias('pu2')
    u_pat = mv_col([(hidu, pu2_w, 'c')], DP, pu2_b, AF.Copy, vecs, "upat")
    u_addr_row = mv_row([(hidu, pu2_w, 'c')], D, pu2_b, AF.Copy,
                        "rowbuf", m_off=DP)

    pl1_ws = load_row(wdram['pl1_w'], 0, 64, wsmall, "wrow")
    pl1_wc = load_small(wdram['pl1_w'], 1, 1, 64)
    hidpl = mv_col([(surp, pl1_ws, 's'), (ctx_col, pl1_wc, 'c')],
                   64, load_bias('pl1'), AF.Gelu, vecs, "hidpl")
    pl2_w = load_small(wdram['pl2_w'], 0, 0, 1, krem=64)
    plr_e = mv_col([(hidpl, pl2_w, 'c')], 1, load_bias('pl2'),
                   AF.Exp, vecs, "plrsp")
    plr = vecs.tile([1, 1], F32, tag="plr")
    nc.vector.tensor_scalar_add(plr, plr_e[0:1, 0:1], 1.0)
    nc.scalar.activation(plr, plr, AF.Ln)
    nc.vector.tensor_single_scalar(plr, plr, 0.1, op=OP.min)
    sc = vecs.tile([1, 1], F32, tag="sc")
    nc.vector.tensor_mul(sc, ws_, plr)
    sc_b = bcast(sc, 1, vecs, "scb")

    d_addr_b = bcast(d_addr_row, DP, vecs, "daddrb")
    scd = vecs.tile([128, DCOL], F32, tag="scd")
    nc.vector.tensor_scalar_mul(scd, d_pat, sc_b)
    wdn_new = bigscr.tile([128, DCOL, DP], F32, tag="bigscr")
    for c in range(DCOL):
        nc.vector.scalar_tensor_tensor(wdn_new[:, c, :], d_addr_b,
                                       scd[:, c:c + 1], wdm[:, c, :],
                                       op0=OP.mult, op1=OP.add)
    nc.sync.dma_start(out=wdn_out.rearrange("(c p) q -> p c q", p=128),
                      in_=wdn_new)

    u_addr_b = bcast(u_addr_row, D, bigscr, "bigscr")
    scu = vecs.tile([128, 1], F32, tag="scu")
    nc.vector.tensor_scalar_mul(scu, u_pat, sc_b)
    wun_new = bigscr.tile([128, D], F32, tag="bigscr")
    nc.vector.scalar_tensor_tensor(wun_new, u_addr_b, scu, wum,
                                   op0=OP.mult, op1=OP.add)
    nc.sync.dma_start(out=wun_out, in_=wun_new)

    # ---- gate ----
    g1_w = load_chunks(wdram['g1_w'], 0, DCOL, DH)
    g1_wo = load_chunks(wdram['g1_w'], D, DCOL, DH)
    g1_wc = load_small(wdram['g1_w'], 2 * D, 1, DH)
    hidg = mv_col([(h_avg, g1_w, 'c'), (output, g1_wo, 'c'),
                   (ctx_col, g1_wc, 'c')],
                  DH, load_bias('g1'), AF.Gelu, vecs, "hidg")
    g2_w = load_small(wdram['g2_w'], 0, 3, 1)
    gate = mv_col([(hidg, g2_w, 'c')], 1, load_bias('g2'),
                  AF.Sigmoid, vecs, "gate")
    g_b = bcast(gate, 1, vecs, "gb")

    c_col = vecs.tile([128, DCOL], F32, tag="ccol")
    nc.vector.tensor_scalar_mul(c_col, output, g_b)
    c_row = col_to_row(c_col, DCOL, "rowbuf")
    c_b = bcast(c_row, D, persist, "cb")

    if ln_affine:
        lnw_b = bcast(load_row(wdram['ln_w'], 0, D, brow, "bias"), D,
                      persist, "lnwb")
        lnb_b = bcast(load_row(wdram['ln_b'], 0, D, brow, "bias"), D,
                      persist, "lnbb")

    # =========================================================
    # PASS 2: h_new = LN(h + c)
    # =========================================================
    for t in range(ST):
        h_t = hpool.tile([128, D], F32, tag="htile")
        nc.sync.dma_start(out=h_t, in_=h_in[t * 128:(t + 1) * 128, :])
        nc.vector.tensor_add(h_t, h_t, c_b)
        st2 = hpool.tile([128, NCH, 6], F32, tag="p2stats")
        for c in range(NCH):
            nc.vector.bn_stats(st2[:, c, :], h_t[:, c * 512:(c + 1) * 512])
        mv2 = hpool.tile([128, 2], F32, tag="p2mv")
        nc.vector.bn_aggr(mv2, st2)
        rstd2 = hpool.tile([128, 1], F32, tag="p2rstd")
        nc.scalar.activation(rstd2, mv2[:, 1:2], AF.Sqrt, bias=eps5)
        nc.vector.reciprocal(rstd2, rstd2)
        nbias2 = hpool.tile([128, 1], F32, tag="p2nbias")
        nc.vector.scalar_tensor_tensor(nbias2, mv2[:, 0:1], -1.0, rstd2,
                                       op0=OP.mult, op1=OP.mult)
        nc.scalar.activation(h_t, h_t, AF.Identity, bias=nbias2, scale=rstd2)
        if ln_affine:
            nc.vector.tensor_mul(h_t, h_t, lnw_b)
            nc.vector.tensor_add(h_t, h_t, lnb_b)
        nc.sync.dma_start(out=h_out[t * 128:(t + 1) * 128, :], in_=h_t)

    stack.close()


# =============================================================
# Host wrapper
# =============================================================
_CACHE = {}


def _get_program(S, ln_affine, sln_affine):
    key = (S, ln_affine, sln_affine)
    if key not in _CACHE:
        _CACHE[key] = build(S, ln_affine, sln_affine)
    return _CACHE[key]


def make_in_maps(h, mem_A, W_down_mod, W_up_mod, prev_h_avg, context, p,
                 ln_affine, sln_affine):
    shared = {'ident': np.eye(128, dtype=np.float32)}
    for name, (K, M) in _WSPECS.items():
        shared[name + '_w'] = np.ascontiguousarray(p[name + '_w'], dtype=np.float32)
        shared[name + '_b'] = np.ascontiguousarray(
            np.asarray(p[name + '_b'], dtype=np.float32).reshape(1, M))
    for name in ['W_K', 'W_V', 'W_down_base', 'W_up_base']:
        shared[name] = np.ascontiguousarray(p[name], dtype=np.float32)
    if sln_affine:
        shared['sln_w'] = np.asarray(p['sln_w'], np.float32).reshape(1, D).copy()
        shared['sln_b'] = np.asarray(p['sln_b'], np.float32).reshape(1, D).copy()
    if ln_affine:
        shared['ln_w'] = np.asarray(p['ln_w'], np.float32).reshape(1, D).copy()
        shared['ln_b'] = np.asarray(p['ln_b'], np.float32).reshape(1, D).copy()

    in_maps = []
    for b in range(h.shape[0]):
        m = dict(shared)
        m['h'] = np.ascontiguousarray(h[b])
        m['mem_A'] = np.ascontiguousarray(mem_A[b])
        m['W_down_mod'] = np.ascontiguousarray(W_down_mod[b])
        m['W_up_mod'] = np.ascontiguousarray(W_up_mod[b])
        m['pha_col'] = np.ascontiguousarray(prev_h_avg[b].reshape(DCOL, 128).T)
        m['ctx_col'] = np.ascontiguousarray(context[b].reshape(1, DC).T)
        in_maps.append(m)
    return in_maps


def kernel(h, mem_A, W_down_mod, W_up_mod, prev_h_avg, context, params):
    from concourse.bass_utils import run_bass_kernel_spmd

    h = np.asarray(h, dtype=np.float32)
    mem_A = np.asarray(mem_A, dtype=np.float32)
    W_down_mod = np.asarray(W_down_mod, dtype=np.float32)
    W_up_mod = np.asarray(W_up_mod, dtype=np.float32)
    prev_h_avg = np.asarray(prev_h_avg, dtype=np.float32)
    context = np.asarray(context, dtype=np.float32)
    p = {k: np.asarray(v, dtype=np.float32) for k, v in params.items()}

    ln_affine = not (np.all(p['ln_w'] == 1.0) and np.all(p['ln_b'] == 0.0))
    sln_affine = not (np.all(p['sln_w'] == 1.0) and np.all(p['sln_b'] == 0.0))

    nc = _get_program(h.shape[1], ln_affine, sln_affine)
    in_maps = make_in_maps(h, mem_A, W_down_mod, W_up_mod, prev_h_avg,
                           context, p, ln_affine, sln_affine)

    res = run_bass_kernel_spmd(nc, in_maps, core_ids=list(range(len(in_maps))))
    outs = res.results
    nb = len(in_maps)
    h_new = np.stack([outs[b]['h_new'] for b in range(nb)])
    mem = np.stack([outs[b]['mem_out'] for b in range(nb)])
    wdn = np.stack([outs[b]['W_down_new'] for b in range(nb)])
    wun = np.stack([outs[b]['W_up_new'] for b in range(nb)])
    return h_new, mem, wdn, wun


# revision 74
# speedup vs baseline: 90.5475x; 90.5475x over previous
"""Trainium2 Bass kernel for the FastNeuron scatter-memory module.

Strategy: pure data-parallel over batch B=8 -> one batch element per
NeuronCore, identical SPMD program on 8 cores.

Per-core program (batch element b):
  pass 1: stream h (4096x2560) once, reduce over S via PE ones-matmul
          -> h_avg.
  middle: the whole surprise/write/read/projection chain as small
          matvecs on PE (column-form: K on partitions, M in chunks of
          128 columns), activations on ACT, elementwise on DVE.
          Weights stream from HBM in chunk-major layout.
  pass 2: stream h again, fused  h_new = LN(h + g*output)  with
          DVE add + bn_stats, ACT normalize, DMA out.

All I/O is f32.  Everything is DMA-bound by design (~206 MB/core).
"""

import contextlib
import math

import numpy as np

import concourse.bacc as bacc
import concourse.bass as bass
import concourse.tile as tile
from concourse import mybir

F32 = mybir.dt.float32
BF = mybir.dt.bfloat16
FP8 = mybir.dt.float8e4
WSCALE = 256.0  # host-side weight scale for fp8 (descaled in ACT copies)
AF = mybir.ActivationFunctionType
OP = mybir.AluOpType

B, S_FULL, D = 8, 4096, 2560
R, DQ, DV, DP, DC, DH = 64, 128, 512, 128, 128, 384
MAX_NORM = 10.0
DCOL = D // 128  # 20 columns in col128 layout
NCH = D // 512   # 5 chunks of 512

# (name, K, M) for every linear layer's weight
_WSPECS = {
    'sp1': (D + DC, DH), 'sp2': (DH, D),
    'su1': (D + DC, DH), 'su2': (DH, 1),
    'wk1': (D + 1 + DC, DH), 'wk2': (DH, R),
    'wv1': (D + 1 + DC, DH), 'wv2': (DH, D),
    'lr1': (1 + DC, DH // 2), 'lr2': (DH // 2, 1),
    'rq': (D + DC, DQ),
    'vup': (DV, D),
    'pd1': (D + 1 + D + DC, DH), 'pd2': (DH, D + DP),
    'pu1': (D + 1 + D + DC, DH), 'pu2': (DH, DP + D),
    'pl1': (1 + DC, 64), 'pl2': (64, 1),
    'g1': (D + D + DC, DH), 'g2': (DH, 1),
}

_WMAXFREE = 2688  # elements per partition per weight-stage tile

# packed-weight registry: key -> (src_param, r0, nk, krem, M, mybir dtype)
_PACK = {}


def _cdiv(a, b):
    return (a + b - 1) // b


def build(S=S_FULL, ln_affine=False, sln_affine=False):
    """Build the per-core Bass program."""
    assert S % 128 == 0
    ST = S // 128

    nc = bacc.Bacc("TRN2", target_bir_lowering=False, debug=False)

    h_in = nc.dram_tensor("h", [S, D], F32, kind="ExternalInput").ap()
    memA_in = nc.dram_tensor("mem_A_pk", [128, D // 128 * R], F32,
                             kind="ExternalInput").ap()
    wdm_in = nc.dram_tensor("W_down_mod", [D, DP], F32, kind="ExternalInput").ap()
    wum_in = nc.dram_tensor("W_up_mod", [DP, D], F32, kind="ExternalInput").ap()
    pha_in = nc.dram_tensor("pha_col", [128, DCOL], BF, kind="ExternalInput").ap()
    ctx_in = nc.dram_tensor("ctx_col", [128, 1], BF, kind="ExternalInput").ap()
    ident_in = nc.dram_tensor("ident", [128, 128], F32, kind="ExternalInput").ap()

    wdram = {}
    for name, (K, M) in _WSPECS.items():
        wdram[name + '_b'] = nc.dram_tensor(name + '_b', [1, M], BF,
                                            kind="ExternalInput").ap()
    wdram['W_up_base'] = nc.dram_tensor('W_up_base', [DP, D], BF,
                                        kind="ExternalInput").ap()
    if sln_affine:
        wdram['sln_w'] = nc.dram_tensor('sln_w', [1, D], F32, kind="ExternalInput").ap()
        wdram['sln_b'] = nc.dram_tensor('sln_b', [1, D], F32, kind="ExternalInput").ap()
    if ln_affine:
        wdram['ln_w'] = nc.dram_tensor('ln_w', [1, D], F32, kind="ExternalInput").ap()
        wdram['ln_b'] = nc.dram_tensor('ln_b', [1, D], F32, kind="ExternalInput").ap()

    h_out = nc.dram_tensor("h_new", [S, D], F32, kind="ExternalOutput").ap()
    mem_out = nc.dram_tensor("mem_out", [D, R], F32, kind="ExternalOutput").ap()
    wdn_out = nc.dram_tensor("W_down_new", [D, DP], F32, kind="ExternalOutput").ap()
    wun_out = nc.dram_tensor("W_up_new", [DP, D], F32, kind="ExternalOutput").ap()

    with tile.TileContext(nc) as tc:
        _body(nc, tc, S, ST, h_in, memA_in, wdm_in, wum_in, pha_in, ctx_in,
              ident_in, wdram, h_out, mem_out, wdn_out, wun_out,
              ln_affine, sln_affine)

    nc.compile()
    return nc


def _body(nc, tc, S, ST, h_in, memA_in, wdm_in, wum_in, pha_in, ctx_in,
          ident_in, wdram, h_out, mem_out, wdn_out, wun_out,
          ln_affine, sln_affine):
    stack = contextlib.ExitStack()
    persist = stack.enter_context(tc.tile_pool(name="persist", bufs=1))
    hpool = stack.enter_context(tc.tile_pool(name="hpool", bufs=6))
    wstream = stack.enter_context(tc.tile_pool(name="wstream", bufs=9))
    wsmall = stack.enter_context(tc.tile_pool(name="wsmall", bufs=3))
    brow = stack.enter_context(tc.tile_pool(name="brow", bufs=2))
    rows = stack.enter_context(tc.tile_pool(name="rows", bufs=2))
    vecs = stack.enter_context(tc.tile_pool(name="vecs", bufs=2))
    bigscr = stack.enter_context(tc.tile_pool(name="bigscr", bufs=3))
    pscol = stack.enter_context(tc.tile_pool(name="pscol", bufs=3, space="PSUM"))
    psrow = stack.enter_context(tc.tile_pool(name="psrow", bufs=1, space="PSUM"))

    # ---------------- persistent small tiles ----------------
    ident = persist.tile([128, 128], F32)
    nc.sync.dma_start(out=ident, in_=ident_in)
    one = ident[0:1, 0:1]  # scalar 1.0

    ones_col = persist.tile([128, 1], F32)
    nc.vector.memset(ones_col, 1.0)
    ones_row = persist.tile([1, 128], F32)
    nc.vector.memset(ones_row, 1.0)

    one_bf = persist.tile([1, 1], BF)
    nc.vector.memset(one_bf, 1.0)

    ctx_col = persist.tile([128, 1], BF)
    nc.sync.dma_start(out=ctx_col, in_=ctx_in)
    pha_col = persist.tile([128, DCOL], BF)
    nc.sync.dma_start(out=pha_col, in_=pha_in)

    eps5 = persist.tile([128, 1], F32)
    nc.vector.memset(eps5, 1e-5)
    eps8 = persist.tile([1, 1], F32)
    nc.vector.memset(eps8, 1e-8)
    neg25 = persist.tile([1, 1], F32)
    nc.vector.memset(neg25, -2.5)
    ln10 = persist.tile([1, 1], F32)
    nc.vector.memset(ln10, 2.302585093)

    # ---------------- helpers ----------------
    def load_chunks(wname, r0, nk, M, krem=0, pool=None, tag="wstream", dt=FP8):
        """Load rows [r0, r0+nk*128+krem) of param `wname`, host-pre-packed
        chunk-major so every DMA is fully contiguous per partition."""
        pool = pool or wstream
        segs = []
        if nk:
            key = f"{wname}_p{r0}"
            _PACK[key] = (wname, r0, nk, 0, M, dt)
            pk = nc.dram_tensor(key, [128, nk * M], dt,
                                kind="ExternalInput").ap()
            G = max(1, _WMAXFREE // M)
            c0 = 0
            while c0 < nk:
                g = min(G, nk - c0)
                t = pool.tile([128, g, M], dt, tag=tag)
                src = pk[:, c0 * M:(c0 + g) * M].rearrange(
                    "p (c m) -> p c m", m=M)
                nc.sync.dma_start(out=t, in_=src)
                segs.append((t, g, 0))
                c0 += g
        if krem:
            key = f"{wname}_q{r0 + nk * 128}"
            _PACK[key] = (wname, r0 + nk * 128, 0, krem, M, dt)
            pk = nc.dram_tensor(key, [krem, M], dt, kind="ExternalInput").ap()
            t = pool.tile([128, 1, M], dt, tag=tag)
            nc.sync.dma_start(out=t[0:krem, 0, :], in_=pk)
            segs.append((t, 1, krem))
        return segs

    def load_small(wname, r0, nk, M, krem=0):
        return load_chunks(wname, r0, nk, M, krem=krem, pool=wsmall, tag="wsm")

    def load_wrow(wname, r0, M):
        key = f"{wname}_q{r0}"
        _PACK[key] = (wname, r0, 0, 1, M, FP8)
        pk = nc.dram_tensor(key, [1, M], FP8, kind="ExternalInput").ap()
        t = wsmall.tile([1, M], FP8, tag="wrow")
        nc.sync.dma_start(out=t, in_=pk)
        return t

    def load_row(w_ap, r0, M, pool, tag, dt=FP8):
        t = pool.tile([1, M], dt, tag=tag)
        nc.sync.dma_start(out=t, in_=w_ap[r0:r0 + 1, :])
        return t

    def load_bias(name):
        return load_row(wdram[name + '_b'], 0, _WSPECS[name][1], brow, "bias",
                        dt=BF)

    def _chunk_ops(parts):
        """Flatten parts into per-K-chunk (wt_slice_fn, x_slice) pairs,
        ordered so weight segments are consumed strictly sequentially."""
        ops = []
        for (x, segs, kind) in parts:
            if kind == 's':
                def fn(lo, w, wt=segs):
                    return wt[0:1, lo:lo + w]
                ops.append((fn, x[0:1, 0:1]))
                continue
            xc = 0
            for (t, g, kr) in segs:
                if kr:
                    def fn(lo, w, t=t, kr=kr):
                        return t[0:kr, 0, lo:lo + w]
                    ops.append((fn, x[0:kr, xc:xc + 1]))
                    xc += 1
                else:
                    for c in range(g):
                        def fn(lo, w, t=t, c=c):
                            return t[:, c, lo:lo + w]
                        ops.append((fn, x[:, xc:xc + 1]))
                        xc += 1
        return ops

    def mv_col(parts, M, bias_row, act, out_pool, tag, m_off=0, act_scale=1.0,
               out_dt=BF):
        """Column-form matvec -> sbuf col tile [128, ceil(M/128)].
        K-chunk-major loop so weight segments stream through few slots."""
        n_mc = _cdiv(M, 128)
        widths = [min(128, M - mc * 128) for mc in range(n_mc)]
        ps = pscol.tile([128, n_mc], F32, tag="colps")
        ops = _chunk_ops(parts)
        total = (len(ops) + (1 if bias_row is not None else 0)) * n_mc
        # start=True zeroes the WHOLE 2KB psum bank, so only the very first
        # matmul of the matvec starts; later columns accumulate onto zeros.
        idx = 0
        for (fn, xs) in ops:
            for mc in range(n_mc):
                lo, w = m_off + mc * 128, widths[mc]
                nc.tensor.matmul(ps[0:w, mc:mc + 1], fn(lo, w), xs,
                                 start=(idx == 0), stop=(idx == total - 1),
                                 skip_group_check=True)
                idx += 1
        if bias_row is not None:
            for mc in range(n_mc):
                lo, w = m_off + mc * 128, widths[mc]
                nc.tensor.matmul(ps[0:w, mc:mc + 1],
                                 bias_row[0:1, lo:lo + w], one_bf,
                                 start=(idx == 0), stop=(idx == total - 1),
                                 skip_group_check=True)
                idx += 1
        out = out_pool.tile([128, n_mc], out_dt, tag=tag)
        sc = act_scale / WSCALE
        if all(w == 128 for w in widths):
            nc.scalar.activation(out, ps, act, scale=sc)
        else:
            for mc in range(n_mc):
                w = widths[mc]
                nc.scalar.activation(out[0:w, mc:mc + 1], ps[0:w, mc:mc + 1],
                                     act, scale=sc)
        return out

    def mv_row(parts, M, bias_row, act, tag, m_off=0, act_scale=1.0):
        """Row-form matvec -> sbuf row tile [1, M]."""
        n_nc = _cdiv(M, 512)
        widths = [min(512, M - i * 512) for i in range(n_nc)]
        ps = psrow.tile([1, M], F32, tag="rowps")
        ops = _chunk_ops(parts)
        total = len(ops) + (1 if bias_row is not None else 0)
        for i, (fn, xs) in enumerate(ops):
            for ncI in range(n_nc):
                lo, w = m_off + ncI * 512, widths[ncI]
                nc.tensor.matmul(ps[0:1, ncI * 512: ncI * 512 + w],
                                 xs, fn(lo, w),
                                 start=(i == 0), stop=(i == total - 1),
                                 skip_group_check=True)
        if bias_row is not None:
            for ncI in range(n_nc):
                lo, w = m_off + ncI * 512, widths[ncI]
                nc.tensor.matmul(ps[0:1, ncI * 512: ncI * 512 + w],
                                 one_bf, bias_row[0:1, lo:lo + w],
                                 start=(total == 1), stop=True,
                                 skip_group_check=True)
        out = rows.tile([1, M], F32, tag=tag)
        nc.scalar.activation(out, ps, act, scale=act_scale / WSCALE)
        return out

    def col_to_row(col, ncols, tag, scale=1.0):
        ps = psrow.tile([1, ncols * 128], F32, tag="rowps")
        for c in range(ncols):
            # 4 chunks of 512B per 2KB bank: start only on each bank's first
            nc.tensor.matmul(ps[0:1, c * 128:(c + 1) * 128], col[:, c:c + 1],
                             ident, start=(c % 4 == 0), stop=(c == ncols - 1),
                             skip_group_check=True)
        out = rows.tile([1, ncols * 128], F32, tag=tag)
        nc.scalar.activation(out, ps, AF.Copy, scale=scale)
        return out

    def row_to_col(row, ncols, out_pool, tag, scale=1.0):
        ps = pscol.tile([128, ncols], F32, tag="colps")
        for c in range(ncols):
            nc.tensor.matmul(ps[:, c:c + 1], row[0:1, c * 128:(c + 1) * 128],
                             one, start=(c == 0), stop=(c == ncols - 1),
                             skip_group_check=True)
        out = out_pool.tile([128, ncols], F32, tag=tag)
        nc.scalar.activation(out, ps, AF.Copy, scale=scale)
        return out

    def bcast(src, n, out_pool, tag):
        """[1, n] -> [128, n] via K=1 PE outer product with a ones column."""
        out = out_pool.tile([128, n], F32, tag=tag)
        for j in range(0, n, 512):
            w = min(512, n - j)
            ps = pscol.tile([128, 512], F32, tag="colps")
            nc.tensor.matmul(ps[:, 0:w], ones_row, src[0:1, j:j + w],
                             start=True, stop=True)
            nc.scalar.activation(out[:, j:j + w], ps[:, 0:w], AF.Copy)
        return out

    # =========================================================
    # PASS 1: sum of h over S
    # =========================================================
    F32R = mybir.dt.float32r
    ones_r = persist.tile([128, 1], F32R)
    nc.vector.tensor_copy(ones_r, ones_col)
    ps_hsum = psrow.tile([1, D], F32, tag="rowps")
    for t in range(ST):
        h_t = hpool.tile([128, D], F32R, tag="htile")
        nc.sync.dma_start(out=h_t, in_=h_in[t * 128:(t + 1) * 128, :].bitcast(F32R))
        for c in range(NCH):
            # float32r: 1 cycle/row on the moving operand (vs 4 for fp32)
            nc.tensor.matmul(ps_hsum[0:1, c * 512:(c + 1) * 512],
                             ones_r,
                             h_t[:, c * 512:(c + 1) * 512],
                             start=(t == 0), stop=(t == ST - 1),
                             skip_group_check=True)

    h_avg_row = rows.tile([1, D], F32, tag="rowbuf")
    nc.scalar.activation(h_avg_row, ps_hsum, AF.Copy, scale=1.0 / S)
    h_avg = row_to_col(h_avg_row, DCOL, persist, "havg")
    h_avg_bf = persist.tile([128, DCOL], BF, tag="havgbf")
    nc.scalar.activation(h_avg_bf, h_avg, AF.Copy)

    # =========================================================
    # MIDDLE
    # =========================================================
    # ---- surprise prediction ----
    sp1_w = load_chunks('sp1_w', 0, DCOL, DH)
    sp1_wc = load_small('sp1_w', D, 1, DH)
    hid = mv_col([(pha_col, sp1_w, 'c'), (ctx_col, sp1_wc, 'c')],
                 DH, load_bias('sp1'), AF.Gelu, vecs, "hid")
    sp2_w = load_chunks('sp2_w', 0, 3, D)
    pred = mv_col([(hid, sp2_w, 'c')], D, load_bias('sp2'), AF.Copy,
                  vecs, "pred", out_dt=F32)

    err = vecs.tile([128, DCOL], BF, tag="err")
    nc.vector.tensor_sub(err, h_avg, pred)

    su1_w = load_chunks('su1_w', 0, DCOL, DH)
    su1_wc = load_small('su1_w', D, 1, DH)
    hid2 = mv_col([(err, su1_w, 'c'), (ctx_col, su1_wc, 'c')],
                  DH, load_bias('su1'), AF.Gelu, vecs, "hid")
    su2_w = load_small('su2_w', 0, 3, 1)
    surprise_t = mv_col([(hid2, su2_w, 'c')], 1, load_bias('su2'),
                        AF.Tanh, vecs, "surpt", act_scale=0.5, out_dt=F32)
    surprise = persist.tile([1, 1], BF, tag="surprise")
    nc.vector.tensor_scalar(surprise, surprise_t[0:1, 0:1], 0.5, 0.5,
                            op0=OP.mult, op1=OP.add)
    surp = surprise[0:1, 0:1]

    # ---- write key / value ----
    wk1_w = load_chunks('wk1_w', 0, DCOL, DH)
    wk1_ws = load_wrow('wk1_w', D, DH)
    wk1_wc = load_small('wk1_w', D + 1, 1, DH)
    wi = [(h_avg_bf, wk1_w, 'c'), (surp, wk1_ws, 's'), (ctx_col, wk1_wc, 'c')]
    hidk = mv_col(wi, DH, load_bias('wk1'), AF.Gelu, vecs, "hidk")
    wk2_w = load_small('wk2_w', 0, 3, R)
    wkey_row = mv_row([(hidk, wk2_w, 'c')], R, load_bias('wk2'),
                      AF.Copy, "rowsm")

    wv1_w = load_chunks('wv1_w', 0, DCOL, DH)
    wv1_ws = load_wrow('wv1_w', D, DH)
    wv1_wc = load_small('wv1_w', D + 1, 1, DH)
    wiv = [(h_avg_bf, wv1_w, 'c'), (surp, wv1_ws, 's'), (ctx_col, wv1_wc, 'c')]
    hidv = mv_col(wiv, DH, load_bias('wv1'), AF.Gelu, vecs, "hidv")
    wv2_w = load_chunks('wv2_w', 0, 3, D)
    wval = mv_col([(hidv, wv2_w, 'c')], D, load_bias('wv2'), AF.Copy,
                  vecs, "wval", out_dt=F32)

    # ---- lr ----
    lr1_ws = load_wrow('lr1_w', 0, DH // 2)
    lr1_wc = load_small('lr1_w', 1, 1, DH // 2)
    hidl = mv_col([(surp, lr1_ws, 's'), (ctx_col, lr1_wc, 'c')],
                  DH // 2, load_bias('lr1'), AF.Gelu, vecs, "hidl")
    lr2_w = load_small('lr2_w', 0, 1, 1, krem=64)
    # softplus(x) = ln(1 + exp(x)) -- no native Softplus LUT set in walrus
    lr_e = mv_col([(hidl, lr2_w, 'c')], 1, load_bias('lr2'),
                  AF.Exp, vecs, "lrsp", out_dt=F32)
    lr = vecs.tile([1, 1], F32, tag="lr")
    nc.vector.tensor_scalar_add(lr, lr_e[0:1, 0:1], 1.0)
    nc.scalar.activation(lr, lr, AF.Ln)
    nc.vector.tensor_single_scalar(lr, lr, 0.1, op=OP.min)

    # ---- mem = mem_A + lr * wval wkey^T ; clip by global norm ----
    lr_b = bcast(lr, 1, vecs, "lrb")
    lrwval = vecs.tile([128, DCOL], F32, tag="lrwval")
    nc.vector.tensor_scalar_mul(lrwval, wval, lr_b)
    wkey_b = bcast(wkey_row, R, vecs, "wkeyb")

    memA = bigscr.tile([128, DCOL, R], F32, tag="bigscr")
    nc.sync.dma_start(out=memA, in_=memA_in.rearrange("p (c r) -> p c r", r=R))
    mem_pre = persist.tile([128, DCOL, R], F32, tag="mempre")
    for c in range(DCOL):
        nc.vector.scalar_tensor_tensor(mem_pre[:, c, :], wkey_b,
                                       lrwval[:, c:c + 1], memA[:, c, :],
                                       op0=OP.mult, op1=OP.add)
    sq_acc = vecs.tile([128, 1], F32, tag="sqacc")
    sq_scr = bigscr.tile([128, DCOL * R], F32, tag="bigscr")
    nc.scalar.activation(sq_scr, mem_pre.rearrange("p c r -> p (c r)"),
                         AF.Square, accum_out=sq_acc)
    ps_n = pscol.tile([128, 1], F32, tag="colps")
    nc.tensor.matmul(ps_n[0:1, 0:1], sq_acc, ones_col, start=True, stop=True)
    lnv = vecs.tile([1, 1], F32, tag="lnv")
    nc.scalar.activation(lnv, ps_n[0:1, 0:1], AF.Ln, bias=eps8)
    mscale = vecs.tile([1, 1], F32, tag="mscale")
    # 10/sqrt(s) = exp(ln10 - 0.5*ln(s))
    nc.scalar.activation(mscale, lnv, AF.Exp, scale=-0.5, bias=ln10)
    nc.vector.tensor_single_scalar(mscale, mscale, 1.0, op=OP.min)
    ms_b = bcast(mscale, 1, vecs, "msb")
    nc.vector.tensor_scalar_mul(mem_pre.rearrange("p c r -> p (c r)"),
                                mem_pre.rearrange("p c r -> p (c r)"), ms_b)
    nc.sync.dma_start(out=mem_out.rearrange("(c p) r -> p c r", p=128),
                      in_=mem_pre)

    # ---- slots = LN(mem^T) ----
    memT = bigscr.tile([R, D], F32, tag="bigscr")
    psT = psrow.tile([R, D], F32, tag="rowps")
    for c in range(DCOL):
        nc.tensor.matmul(psT[0:R, c * 128:(c + 1) * 128], mem_pre[:, c, :],
                         ident, start=(c % 4 == 0), stop=(c == DCOL - 1),
                         skip_group_check=True)
    nc.scalar.activation(memT, psT, AF.Copy)
    stats = vecs.tile([R, NCH, 6], F32, tag="slnstats")
    for c in range(NCH):
        nc.vector.bn_stats(stats[:, c, :], memT[:, c * 512:(c + 1) * 512])
    mv_ = vecs.tile([R, 2], F32, tag="slnmv")
    nc.vector.bn_aggr(mv_, stats)
    s_rstd = vecs.tile([R, 1], F32, tag="srstd")
    nc.scalar.activation(s_rstd, mv_[:, 1:2], AF.Ln, bias=eps5[0:R, 0:1])
    nc.scalar.activation(s_rstd, s_rstd, AF.Exp, scale=-0.5)
    s_nbias = vecs.tile([R, 1], F32, tag="snbias")
    nc.vector.scalar_tensor_tensor(s_nbias, mv_[:, 0:1], -1.0, s_rstd,
                                   op0=OP.mult, op1=OP.mult)
    slots = bigscr.tile([R, D], F32, tag="bigscr")
    nc.scalar.activation(slots, memT, AF.Identity, bias=s_nbias, scale=s_rstd)
    if sln_affine:
        slw = bcast(load_row(wdram['sln_w'], 0, D, brow, "biasf", dt=F32), D,
                    bigscr, "bigscr")
        slb = bcast(load_row(wdram['sln_b'], 0, D, brow, "biasf", dt=F32), D,
                    bigscr, "bigscr")
        nc.vector.tensor_mul(slots, slots, slw[0:R, :])
        nc.vector.tensor_add(slots, slots, slb[0:R, :])

    # W_down_eff / W_up_eff depend only on inputs; compute here on the
    # otherwise-idle DVE so they are off the attention critical chain
    wdb = load_chunks('W_down_base', 0, DCOL, DP, dt=BF)
    wdm = bigscr.tile([128, DCOL, DP], F32, tag="bigscr")
    nc.sync.dma_start(out=wdm, in_=wdm_in.rearrange("(c p) q -> p c q", p=128))
    assert len(wdb) == 1
    wdeff = persist.tile([128, DCOL, DP], BF, tag="wdeff")
    nc.vector.tensor_add(wdeff.rearrange("p c q -> p (c q)"),
                         wdb[0][0].rearrange("p c q -> p (c q)"),
                         wdm.rearrange("p c q -> p (c q)"))

    slotsT = persist.tile([128, DCOL, R], BF, tag="slotsT")
    psT2 = psrow.tile([128, DCOL, R], F32, tag="rowps")
    for c in range(DCOL):
        nc.tensor.matmul(psT2[:, c, :], slots[:, c * 128:(c + 1) * 128],
                         ident[0:R, 0:R], start=(c % 8 == 0),
                         stop=(c == DCOL - 1), skip_group_check=True)
    nc.scalar.activation(slotsT, psT2, AF.Copy)

    wub = bigscr.tile([128, D], BF, tag="bigscr")
    nc.sync.dma_start(out=wub, in_=wdram['W_up_base'])
    wum = bigscr.tile([128, D], F32, tag="bigscr")
    nc.sync.dma_start(out=wum, in_=wum_in)
    wueff = persist.tile([128, D], BF, tag="wueff")
    nc.vector.tensor_add(wueff, wub, wum)

    # ---- attention read ----
    wk_w = load_chunks('W_K', 0, DCOL, DQ)
    ps_k = pscol.tile([128, R], F32, tag="colps")
    ci = 0
    for (t_, g, kr) in wk_w:
        for c in range(g):
            nc.tensor.matmul(ps_k, t_[:, c, :], slotsT[:, ci, :],
                             start=(ci == 0), stop=(ci == DCOL - 1))
            ci += 1
    keysT = vecs.tile([128, R], BF, tag="keysT")
    nc.scalar.activation(keysT, ps_k, AF.Copy,
                         scale=1.0 / math.sqrt(DQ) / WSCALE)

    rq_w = load_chunks('rq_w', 0, DCOL, DQ)
    rq_wc = load_small('rq_w', D, 1, DQ)
    query = mv_col([(h_avg_bf, rq_w, 'c'), (ctx_col, rq_wc, 'c')],
                   DQ, load_bias('rq'), AF.Copy, vecs, "query")

    ps_s = psrow.tile([1, R], F32, tag="rowps")
    nc.tensor.matmul(ps_s, query, keysT, start=True, stop=True)
    smax = vecs.tile([1, 1], F32, tag="smax")
    nc.vector.reduce_max(smax, ps_s, axis=mybir.AxisListType.X)
    negmax = vecs.tile([1, 1], F32, tag="negmax")
    nc.vector.tensor_scalar_mul(negmax, smax, -1.0)
    esum = vecs.tile([1, 1], F32, tag="esum")
    attn_row = rows.tile([1, R], F32, tag="rowsm")
    nc.scalar.activation(attn_row, ps_s, AF.Exp, bias=negmax, accum_out=esum)
    einv = vecs.tile([1, 1], F32, tag="einv")
    nc.vector.reciprocal(einv, esum)
    nc.vector.tensor_scalar_mul(attn_row, attn_row, einv)
    ps_a = pscol.tile([128, 1], F32, tag="colps")
    nc.tensor.matmul(ps_a[0:R, 0:1], attn_row, one, start=True, stop=True)
    attn_col = vecs.tile([R, 1], BF, tag="attncol")
    nc.scalar.activation(attn_col, ps_a[0:R, 0:1], AF.Copy)

    wv_w = load_chunks('W_V', 0, DCOL, DV)
    ps_v = pscol.tile([R, DV], F32, tag="colps")
    ci = 0
    for (t_, g, kr) in wv_w:
        for c in range(g):
            nc.tensor.matmul(ps_v, slotsT[:, ci, :], t_[:, c, :],
                             start=(ci == 0), stop=(ci == DCOL - 1))
            ci += 1
    vals = bigscr.tile([R, DV], BF, tag="bigscr")
    nc.scalar.activation(vals, ps_v, AF.Copy, scale=1.0 / WSCALE)

    ps_mr = pscol.tile([128, DV // 128], F32, tag="colps")
    for dc in range(DV // 128):
        nc.tensor.matmul(ps_mr[:, dc:dc + 1], vals[:, dc * 128:(dc + 1) * 128],
                         attn_col, start=(dc == 0), stop=(dc == DV // 128 - 1),
                         skip_group_check=True)
    mr_dv = vecs.tile([128, DV // 128], BF, tag="mrdv")
    nc.scalar.activation(mr_dv, ps_mr, AF.Copy)

    # prefetch the first pass-2 h tiles into the DMA-quiet attention window
    p2_pre = []
    for t in range(min(2, ST)):
        h_t = hpool.tile([128, D], F32, tag="htile")
        nc.sync.dma_start(out=h_t, in_=h_in[t * 128:(t + 1) * 128, :])
        p2_pre.append(h_t)

    vup_w = load_chunks('vup_w', 0, 4, D)
    mem_read = mv_col([(mr_dv, vup_w, 'c')], D, load_bias('vup'),
                      AF.Copy, persist, "memread", out_dt=F32)
    mem_read_bf = persist.tile([128, DCOL], BF, tag="memreadbf")
    nc.scalar.activation(mem_read_bf, mem_read, AF.Copy)

    # ---- modulated projection ----
    ps_dn = pscol.tile([128, 1], F32, tag="colps")
    for c in range(DCOL):
        nc.tensor.matmul(ps_dn, wdeff[:, c, :], mem_read_bf[:, c:c + 1],
                         start=(c == 0), stop=(c == DCOL - 1))
    down = vecs.tile([128, 1], BF, tag="down")
    nc.scalar.activation(down, ps_dn, AF.Gelu)

    ps_pr = pscol.tile([128, DCOL], F32, tag="colps")
    for c in range(DCOL):
        nc.tensor.matmul(ps_pr[:, c:c + 1], wueff[:, c * 128:(c + 1) * 128],
                         down, start=(c == 0), stop=(c == DCOL - 1),
                         skip_group_check=True)
    proj = vecs.tile([128, DCOL], F32, tag="proj")
    nc.scalar.activation(proj, ps_pr, AF.Copy)

    output = persist.tile([128, DCOL], F32, tag="output")
    nc.vector.scalar_tensor_tensor(output, mem_read, 0.5, proj,
                                   op0=OP.mult, op1=OP.add)
    output_bf = persist.tile([128, DCOL], BF, tag="outputbf")
    nc.scalar.activation(output_bf, output, AF.Copy)

    # ---- projection weight write ----
    ws_ = vecs.tile([1, 1], F32, tag="ws")
    nc.scalar.activation(ws_, surp, AF.Tanh, bias=neg25, scale=5.0)
    nc.vector.tensor_scalar(ws_, ws_, 0.5, 0.5, op0=OP.mult, op1=OP.add)

    pd1_w = load_chunks('pd1_w', 0, DCOL, DH)
    pd1_ws = load_wrow('pd1_w', D, DH)
    pd1_wm = load_chunks('pd1_w', D + 1, DCOL, DH)
    pd1_wc = load_small('pd1_w', 2 * D + 1, 1, DH)
    pwi_d = [(h_avg_bf, pd1_w, 'c'), (surp, pd1_ws, 's'),
             (mem_read_bf, pd1_wm, 'c'), (ctx_col, pd1_wc, 'c')]
    hidp = mv_col(pwi_d, DH, load_bias('pd1'), AF.Gelu, vecs, "hidp")
    pd2_w = load_chunks('pd2_w', 0, 3, D + DP)
    pd2_b = load_bias('pd2')
    d_pat = mv_col([(hidp, pd2_w, 'c')], D, pd2_b, AF.Copy, vecs, "dpat",
                   out_dt=F32)
    d_addr_row = mv_row([(hidp, pd2_w, 'c')], DP, pd2_b, AF.Copy,
                        "rowsm", m_off=D)

    pu1_w = load_chunks('pu1_w', 0, DCOL, DH)
    pu1_ws = load_wrow('pu1_w', D, DH)
    pu1_wm = load_chunks('pu1_w', D + 1, DCOL, DH)
    pu1_wc = load_small('pu1_w', 2 * D + 1, 1, DH)
    pwi_u = [(h_avg_bf, pu1_w, 'c'), (surp, pu1_ws, 's'),
             (mem_read_bf, pu1_wm, 'c'), (ctx_col, pu1_wc, 'c')]
    hidu = mv_col(pwi_u, DH, load_bias('pu1'), AF.Gelu, vecs, "hidu")
    pu2_w = load_chunks('pu2_w', 0, 3, DP + D)
    pu2_b = load_bias('pu2')
    u_pat = mv_col([(hidu, pu2_w, 'c')], DP, pu2_b, AF.Copy, vecs, "upat",
                   out_dt=F32)
    u_addr_row = mv_row([(hidu, pu2_w, 'c')], D, pu2_b, AF.Copy,
                        "rowbuf", m_off=DP)

    pl1_ws = load_wrow('pl1_w', 0, 64)
    pl1_wc = load_small('pl1_w', 1, 1, 64)
    hidpl = mv_col([(surp, pl1_ws, 's'), (ctx_col, pl1_wc, 'c')],
                   64, load_bias('pl1'), AF.Gelu, vecs, "hidpl")
    pl2_w = load_small('pl2_w', 0, 0, 1, krem=64)
    plr_e = mv_col([(hidpl, pl2_w, 'c')], 1, load_bias('pl2'),
                   AF.Exp, vecs, "plrsp", out_dt=F32)
    plr = vecs.tile([1, 1], F32, tag="plr")
    nc.vector.tensor_scalar_add(plr, plr_e[0:1, 0:1], 1.0)
    nc.scalar.activation(plr, plr, AF.Ln)
    nc.vector.tensor_single_scalar(plr, plr, 0.1, op=OP.min)
    sc = vecs.tile([1, 1], F32, tag="sc")
    nc.vector.tensor_mul(sc, ws_, plr)
    sc_b = bcast(sc, 1, vecs, "scb")

    d_addr_b = bcast(d_addr_row, DP, vecs, "daddrb")
    scd = vecs.tile([128, DCOL], F32, tag="scd")
    nc.vector.tensor_scalar_mul(scd, d_pat, sc_b)
    wdn_new = bigscr.tile([128, DCOL, DP], F32, tag="bigscr")
    for c in range(DCOL):
        nc.vector.scalar_tensor_tensor(wdn_new[:, c, :], d_addr_b,
                                       scd[:, c:c + 1], wdm[:, c, :],
                                       op0=OP.mult, op1=OP.add)
    nc.sync.dma_start(out=wdn_out.rearrange("(c p) q -> p c q", p=128),
                      in_=wdn_new)

    u_addr_b = bcast(u_addr_row, D, bigscr, "bigscr")
    scu = vecs.tile([128, 1], F32, tag="scu")
    nc.vector.tensor_scalar_mul(scu, u_pat, sc_b)
    wun_new = bigscr.tile([128, D], F32, tag="bigscr")
    nc.vector.scalar_tensor_tensor(wun_new, u_addr_b, scu, wum,
                                   op0=OP.mult, op1=OP.add)
    nc.sync.dma_start(out=wun_out, in_=wun_new)

    # ---- gate ----
    g1_w = load_chunks('g1_w', 0, DCOL, DH)
    g1_wo = load_chunks('g1_w', D, DCOL, DH)
    g1_wc = load_small('g1_w', 2 * D, 1, DH)
    hidg = mv_col([(h_avg_bf, g1_w, 'c'), (output_bf, g1_wo, 'c'),
                   (ctx_col, g1_wc, 'c')],
                  DH, load_bias('g1'), AF.Gelu, vecs, "hidg")
    g2_w = load_small('g2_w', 0, 3, 1)
    gate = mv_col([(hidg, g2_w, 'c')], 1, load_bias('g2'),
                  AF.Tanh, vecs, "gate", act_scale=0.5, out_dt=F32)
    nc.vector.tensor_scalar(gate[0:1, 0:1], gate[0:1, 0:1], 0.5, 0.5,
                            op0=OP.mult, op1=OP.add)
    g_b = bcast(gate, 1, vecs, "gb")

    c_col = vecs.tile([128, DCOL], F32, tag="ccol")
    nc.vector.tensor_scalar_mul(c_col, output, g_b)
    c_row = col_to_row(c_col, DCOL, "rowbuf")
    c_b = bcast(c_row, D, persist, "cb")

    if ln_affine:
        lnw_b = bcast(load_row(wdram['ln_w'], 0, D, brow, "biasf", dt=F32), D,
                      persist, "lnwb")
        lnb_b = bcast(load_row(wdram['ln_b'], 0, D, brow, "biasf", dt=F32), D,
                      persist, "lnbb")

    # =========================================================
    # PASS 2: h_new = LN(h + c)
    # =========================================================
    for t in range(ST):
        if t < len(p2_pre):
            h_t = p2_pre[t]
        else:
            h_t = hpool.tile([128, D], F32, tag="htile")
            nc.sync.dma_start(out=h_t, in_=h_in[t * 128:(t + 1) * 128, :])
        nc.vector.tensor_add(h_t, h_t, c_b)
        st2 = hpool.tile([128, NCH, 6], F32, tag="p2stats")
        for c in range(NCH):
            nc.vector.bn_stats(st2[:, c, :], h_t[:, c * 512:(c + 1) * 512])
        mv2 = hpool.tile([128, 2], F32, tag="p2mv")
        nc.vector.bn_aggr(mv2, st2)
        rstd2 = hpool.tile([128, 1], F32, tag="p2rstd")
        nc.scalar.activation(rstd2, mv2[:, 1:2], AF.Sqrt, bias=eps5)
        nc.vector.reciprocal(rstd2, rstd2)
        nbias2 = hpool.tile([128, 1], F32, tag="p2nbias")
        nc.vector.scalar_tensor_tensor(nbias2, mv2[:, 0:1], -1.0, rstd2,
                                       op0=OP.mult, op1=OP.mult)
        nc.scalar.activation(h_t, h_t, AF.Identity, bias=nbias2, scale=rstd2)
        if ln_affine:
            nc.vector.tensor_mul(h_t, h_t, lnw_b)
            nc.vector.tensor_add(h_t, h_t, lnb_b)
        nc.sync.dma_start(out=h_out[t * 128:(t + 1) * 128, :], in_=h_t)

    stack.close()


# =============================================================
# Host wrapper
# =============================================================
_CACHE = {}


def _get_program(S, ln_affine, sln_affine):
    key = (S, ln_affine, sln_affine)
    if key not in _CACHE:
        _CACHE[key] = build(S, ln_affine, sln_affine)
    return _CACHE[key]


def make_in_maps(h, mem_A, W_down_mod, W_up_mod, prev_h_avg, context, p,
                 ln_affine, sln_affine):
    import ml_dtypes
    bf16 = np.dtype(ml_dtypes.bfloat16)
    fp8 = np.dtype(ml_dtypes.float8_e4m3)
    ws = np.float32(WSCALE)
    shared = {'ident': np.eye(128, dtype=np.float32)}
    for name, (K, M) in _WSPECS.items():
        shared[name + '_b'] = np.ascontiguousarray(
            np.asarray(p[name + '_b'], dtype=np.float32).reshape(1, M) * ws
        ).astype(bf16)
    shared['W_up_base'] = np.ascontiguousarray(p['W_up_base'], dtype=bf16)
    for key, (wname, r0, nk, krem, M, dt) in _PACK.items():
        W = np.asarray(p[wname], np.float32)
        if dt == FP8:
            npdt, sc = fp8, ws
        elif dt == BF:
            npdt, sc = bf16, np.float32(1.0)
        else:
            npdt, sc = np.float32, np.float32(1.0)
        if nk:
            a = (W[r0:r0 + nk * 128] * sc).reshape(nk, 128, M)
            a = a.transpose(1, 0, 2).reshape(128, nk * M)
        else:
            a = W[r0:r0 + krem] * sc
        shared[key] = np.ascontiguousarray(a).astype(npdt)
    if sln_affine:
        shared['sln_w'] = np.asarray(p['sln_w'], np.float32).reshape(1, D).copy()
        shared['sln_b'] = np.asarray(p['sln_b'], np.float32).reshape(1, D).copy()
    if ln_affine:
        shared['ln_w'] = np.asarray(p['ln_w'], np.float32).reshape(1, D).copy()
        shared['ln_b'] = np.asarray(p['ln_b'], np.float32).reshape(1, D).copy()

    in_maps = []
    for b in range(h.shape[0]):
        m = dict(shared)
        m['h'] = np.ascontiguousarray(h[b])
        m['mem_A_pk'] = np.ascontiguousarray(
            mem_A[b].reshape(DCOL, 128, R).transpose(1, 0, 2).reshape(
                128, DCOL * R))
        m['W_down_mod'] = np.ascontiguousarray(W_down_mod[b])
        m['W_up_mod'] = np.ascontiguousarray(W_up_mod[b])
        m['pha_col'] = np.ascontiguousarray(
            prev_h_avg[b].reshape(DCOL, 128).T).astype(bf16)
        m['ctx_col'] = np.ascontiguousarray(
            context[b].reshape(1, DC).T).astype(bf16)
        in_maps.append(m)
    return in_maps


def kernel(h, mem_A, W_down_mod, W_up_mod, prev_h_avg, context, params):
    from concourse.bass_utils import run_bass_kernel_spmd

    h = np.asarray(h, dtype=np.float32)
    mem_A = np.asarray(mem_A, dtype=np.float32)
    W_down_mod = np.asarray(W_down_mod, dtype=np.float32)
    W_up_mod = np.asarray(W_up_mod, dtype=np.float32)
    prev_h_avg = np.asarray(prev_h_avg, dtype=np.float32)
    context = np.asarray(context, dtype=np.float32)
    p = {k: np.asarray(v, dtype=np.float32) for k, v in params.items()}

    ln_affine = not (np.all(p['ln_w'] == 1.0) and np.all(p['ln_b'] == 0.0))
    sln_affine = not (np.all(p['sln_w'] == 1.0) and np.all(p['sln_b'] == 0.0))

    nc = _get_program(h.shape[1], ln_affine, sln_affine)
    in_maps = make_in_maps(h, mem_A, W_down_mod, W_up_mod, prev_h_avg,
                           context, p, ln_affine, sln_affine)

    res = run_bass_kernel_spmd(nc, in_maps, core_ids=list(range(len(in_maps))))
    outs = res.results
    nb = len(in_maps)
    h_new = np.stack([outs[b]['h_new'] for b in range(nb)])
    mem = np.stack([outs[b]['mem_out'] for b in range(nb)])
    wdn = np.stack([outs[b]['W_down_new'] for b in range(nb)])
    wun = np.stack([outs[b]['W_up_new'] for b in range(nb)])
    return h_new, mem, wdn, wun
